# revision 1
# baseline (speedup 1.0000x reference)
"""Trainium2 Bass kernel for the fused soft-logic-gate layer.

Reference computation:
    pa = softmax(wa, axis=1); pb = softmax(wb, axis=1); pt = softmax(wt, axis=0)
    A = pa @ x; B = pb @ x
    out = sum_g pt[g,:,None] * gate_g(A, B)        (16 soft logic gates)

Every gate is affine in {1, A, B, A*B}, so the 16-gate table collapses to
    out = c0 + cA*A + cB*B + cAB*(A*B)
with four per-row coefficient vectors derived from pt.  Folding the softmax
denominators of wa/wb into those coefficients lets the matmuls run on the raw
exp() weights, and factoring
    out = (A + u) * (cAB*B + cA) + w,   u = cB/cAB,  w = c0 - cA*u
leaves one ACT op + two DVE ops per tile.  The device work is two
[256,256]x[256,b] float32r (TF32) matmuls plus that elementwise pass —
memory-bound on streaming x in and out once.

Sharding: batch axis of x split evenly across 8 NeuronCores (data parallel),
weights replicated.
"""

import os
import sys

for _p in ("/opt/trn_rl_repo",):
    if _p not in sys.path and os.path.isdir(_p):
        sys.path.insert(0, _p)

import numpy as np

SIZE = 256
PREV = 256
BATCH = 32768
N_CORES = 8
BSH = BATCH // N_CORES  # per-core batch shard
CH = 1024               # epilogue chunk width (2 PSUM banks)
NCH = BSH // CH
P = 128

# constants blob layout (f32, [128, 390]):
#   [:, 0:128]     identity
#   [:, 128]       ones column
#   [:16, 129:134] sign matrix [16, 5] (cols: sum, c0, cA, cB, cAB)
#   [:16, 134:390] wt [16, 256]
BLOB_W = 390

_CACHE = {}


def _sign_matrix() -> np.ndarray:
    """[16,5] f32 columns: [colsum, c0, cA, cB, cAB] — gate-table
    coefficients of {1, A, B, A*B} preceded by the softmax denominator."""
    S = np.zeros((16, 5), dtype=np.float32)
    S[:, 0] = 1.0
    S[8:16, 1] = 1.0
    for g in (2, 3, 6, 7):
        S[g, 2] += 1.0
    for g in (8, 9, 12, 13):
        S[g, 2] -= 1.0
    for g in (4, 5, 6, 7):
        S[g, 3] += 1.0
    for g in (8, 9, 10, 11):
        S[g, 3] -= 1.0
    for g, v in {1: 1, 2: -1, 4: -1, 6: -2, 7: -1, 8: 1, 9: 2, 11: 1, 13: 1, 14: -1}.items():
        S[g, 4] = v
    return S


def _build_bass():
    import concourse.bacc as bacc
    import concourse.tile as tile
    import concourse.mybir as mybir

    f32 = mybir.dt.float32
    f32r = mybir.dt.float32r
    Act = mybir.ActivationFunctionType
    Alu = mybir.AluOpType

    nc = bacc.Bacc(trn_type="TRN2", target_bir_lowering=False, debug=False,
                   num_devices=N_CORES)

    xs_d = nc.dram_tensor("xs", [PREV, BSH], f32r, kind="ExternalInput").ap()
    wa_d = nc.dram_tensor("wa", [SIZE, PREV], f32, kind="ExternalInput").ap()
    wb_d = nc.dram_tensor("wb", [SIZE, PREV], f32, kind="ExternalInput").ap()
    bl_d = nc.dram_tensor("blob", [P, BLOB_W], f32, kind="ExternalInput").ap()
    out_d = nc.dram_tensor("out", [SIZE, BSH], f32, kind="ExternalOutput").ap()

    # [p, k/m, b] views for single-DMA transfers
    xs_v = xs_d.rearrange("(k p) b -> p k b", p=P)
    wa_v = wa_d.rearrange("(m p) c -> p m c", p=P)
    wb_v = wb_d.rearrange("(m p) c -> p m c", p=P)

    with tile.TileContext(nc) as tc:
        with tc.tile_pool(name="consts", bufs=1) as consts, \
             tc.tile_pool(name="weights", bufs=1) as weights, \
             tc.tile_pool(name="coefs", bufs=1) as coefs, \
             tc.tile_pool(name="xp", bufs=4) as xp:

            blob = consts.tile([P, BLOB_W], f32)
            nc.sync.dma_start(out=blob[:], in_=bl_d[:])
            ident = blob[:, 0:128]
            smat = blob[:16, 129:134]
            wts = blob[:16, 134:390]

            # tiny early Exp forces the ACT table load off the critical path
            dummy = consts.tile([1, 1], f32)
            nc.scalar.activation(out=dummy[:], in_=blob[0:1, 128:129], func=Act.Exp)

            wa_sb = consts.tile([P, 2, PREV], f32)
            nc.sync.dma_start(out=wa_sb[:], in_=wa_v[:])
            wb_sb = consts.tile([P, 2, PREV], f32)
            nc.sync.dma_start(out=wb_sb[:], in_=wb_v[:])

            # prefetch the first x chunks
            xtiles = []
            for n in range(2):
                xt = xp.tile([P, 2, CH], f32r, tag="x", name=f"x{n}")
                nc.sync.dma_start(out=xt[:], in_=xs_v[:, :, n * CH:(n + 1) * CH])
                xtiles.append(xt)

            # Transposed exp(weights), float32r, live for the whole kernel:
            # eaT[p] is [128(prev-block p), 256(size)].
            eaT = [weights.tile([P, SIZE], f32r, tag=f"eaT{p}", name=f"eaT{p}") for p in range(2)]
            ebT = [weights.tile([P, SIZE], f32r, tag=f"ebT{p}", name=f"ebT{p}") for p in range(2)]

            # [128,2] coefficient tiles (m as free dim):
            cA2 = coefs.tile([P, 2], f32, tag="cA2")
            cAB2 = coefs.tile([P, 2], f32, tag="cAB2")
            cU2 = coefs.tile([P, 2], f32, tag="cU2")
            cW2 = coefs.tile([P, 2], f32, tag="cW2")

            # ---- weight preprocessing ----
            with tc.tile_pool(name="prep", bufs=2) as prep, \
                 tc.tile_pool(name="prep_ps", bufs=3, space="PSUM") as prep_ps, \
                 tc.tile_pool(name="coef_ps", bufs=1, space="PSUM") as coef_ps:

                # pt-coefficient path first: it only needs the blob
                ept = prep.tile([16, SIZE], f32, tag="ept")
                nc.scalar.activation(out=ept[:], in_=wts, func=Act.Exp)
                cps = coef_ps.tile([P, 10], f32, tag="cps")
                for m in range(2):
                    nc.tensor.matmul(cps[:, m * 5:(m + 1) * 5],
                                     ept[:, m * P:(m + 1) * P], smat,
                                     start=True, stop=True)
                cpsv = cps[:].rearrange("p (m c) -> p c m", m=2)
                rpt2 = prep.tile([P, 2], f32, tag="rpt2")
                nc.vector.reciprocal(out=rpt2[:], in_=cpsv[:, 0, :])
                rcabn = prep.tile([P, 2], f32, tag="rcabn")
                nc.vector.reciprocal(out=rcabn[:], in_=cpsv[:, 4, :])

                # exp in natural layout (one wide ACT op per weight), row sums
                # on DVE, then PE-transpose each block and copy out as float32r
                rsa = prep.tile([P, 2], f32, tag="rsa")
                rsb = prep.tile([P, 2], f32, tag="rsb")
                for w_sb, eT, rs, nm in ((wa_sb, eaT, rsa, "a"), (wb_sb, ebT, rsb, "b")):
                    e_nat = prep.tile([P, 2, PREV], f32, tag=f"e{nm}", name=f"e{nm}")
                    nc.scalar.activation(out=e_nat[:], in_=w_sb[:], func=Act.Exp)
                    for m in range(2):
                        nc.vector.tensor_reduce(out=rs[:, m:m + 1], in_=e_nat[:, m, :],
                                                axis=mybir.AxisListType.X, op=Alu.add)
                    for m in range(2):
                        for p in range(2):
                            tp = prep_ps.tile([P, P], f32, tag="tps", name=f"tp{nm}{m}{p}")
                            nc.tensor.transpose(tp[:], e_nat[:, m, p * P:(p + 1) * P], ident)
                            nc.scalar.copy(out=eT[p][:, m * P:(m + 1) * P], in_=tp[:])

                ra2 = prep.tile([P, 2], f32, tag="ra2")
                nc.vector.reciprocal(out=ra2[:], in_=rsa[:])
                rb2 = prep.tile([P, 2], f32, tag="rb2")
                nc.vector.reciprocal(out=rb2[:], in_=rsb[:])

                # batched [128,2] coefficient chain:
                h2 = prep.tile([P, 2], f32, tag="h2")
                nc.vector.tensor_tensor(out=h2[:], in0=rpt2[:], in1=ra2[:], op=Alu.mult)
                nc.vector.tensor_tensor(out=cA2[:], in0=cpsv[:, 2, :], in1=h2[:], op=Alu.mult)
                g2 = prep.tile([P, 2], f32, tag="g2")
                nc.vector.tensor_tensor(out=g2[:], in0=h2[:], in1=rb2[:], op=Alu.mult)
                nc.vector.tensor_tensor(out=cAB2[:], in0=cpsv[:, 4, :], in1=g2[:], op=Alu.mult)

                # u = cBn * sa / cABn ;  w = c0n*rpt - cA*u
                u2a = prep.tile([P, 2], f32, tag="u2a")
                nc.vector.tensor_tensor(out=u2a[:], in0=cpsv[:, 3, :], in1=rcabn[:], op=Alu.mult)
                nc.vector.tensor_tensor(out=cU2[:], in0=u2a[:], in1=rsa[:], op=Alu.mult)
                c02 = prep.tile([P, 2], f32, tag="c02")
                nc.vector.tensor_tensor(out=c02[:], in0=cpsv[:, 1, :], in1=rpt2[:], op=Alu.mult)
                t2 = prep.tile([P, 2], f32, tag="t2")
                nc.vector.tensor_tensor(out=t2[:], in0=cA2[:], in1=cU2[:], op=Alu.mult)
                nc.vector.tensor_tensor(out=cW2[:], in0=c02[:], in1=t2[:], op=Alu.subtract)

            # ---- main loop ----
            with tc.tile_pool(name="ep", bufs=3) as ep, \
                 tc.tile_pool(name="mm_ps", bufs=2, space="PSUM") as mm_ps:
                for n in range(NCH):
                    if n + 2 < NCH:
                        xt = xp.tile([P, 2, CH], f32r, tag="x", name=f"x{n+2}")
                        nc.sync.dma_start(out=xt[:], in_=xs_v[:, :, (n + 2) * CH:(n + 3) * CH])
                        xtiles.append(xt)
                    xk = xtiles[n]
                    for m in range(2):
                        a_ps = mm_ps.tile([P, CH], f32, tag="A", name=f"A{n}{m}")
                        b_ps = mm_ps.tile([P, CH], f32, tag="B", name=f"B{n}{m}")
                        for ps_t, eT in ((a_ps, eaT), (b_ps, ebT)):
                            for k in range(2):
                                for s in range(CH // 512):
                                    sl = slice(s * 512, (s + 1) * 512)
                                    nc.tensor.matmul(ps_t[:, sl],
                                                     eT[k][:, m * P:(m + 1) * P],
                                                     xk[:, k, sl],
                                                     start=(k == 0), stop=(k == 1))
                        # out = (A + u) * (cAB*B + cA) + w
                        s_sb = ep.tile([P, CH], f32, tag="s", name=f"s{n}{m}")
                        nc.scalar.activation(out=s_sb[:], in_=b_ps[:], func=Act.Identity,
                                             scale=cAB2[:, m:m + 1], bias=cA2[:, m:m + 1])
                        p_sb = ep.tile([P, CH], f32, tag="p", name=f"p{n}{m}")
                        nc.vector.scalar_tensor_tensor(out=p_sb[:], in0=a_ps[:],
                                                       scalar=cU2[:, m:m + 1], in1=s_sb[:],
                                                       op0=Alu.add, op1=Alu.mult)
                        o_sb = ep.tile([P, CH], f32, tag="o", name=f"o{n}{m}")
                        nc.vector.tensor_scalar_add(o_sb[:], p_sb[:], cW2[:, m:m + 1])
                        if n == NCH - 1 and m == 1:
                            hw = CH // 2
                            for h in range(2):
                                nc.sync.dma_start(
                                    out=out_d[m * P:(m + 1) * P,
                                              n * CH + h * hw:n * CH + (h + 1) * hw],
                                    in_=o_sb[:, h * hw:(h + 1) * hw])
                        else:
                            nc.sync.dma_start(out=out_d[m * P:(m + 1) * P, n * CH:(n + 1) * CH],
                                              in_=o_sb[:])

    nc.compile()
    return nc


def _get_nc():
    if "nc" not in _CACHE:
        _CACHE["nc"] = _build_bass()
    return _CACHE["nc"]


def _make_blob(wt: np.ndarray) -> np.ndarray:
    blob = np.zeros((P, BLOB_W), dtype=np.float32)
    blob[:, 0:128] = np.eye(P, dtype=np.float32)
    blob[:, 128] = 1.0
    blob[:16, 129:134] = _sign_matrix()
    blob[:16, 134:390] = wt
    return blob


def _run(x, wa, wb, wt, trace=False, **spmd_kwargs):
    from concourse import bass_utils

    nc = _get_nc()
    x = np.ascontiguousarray(np.asarray(x, dtype=np.float32))
    wa = np.ascontiguousarray(np.asarray(wa, dtype=np.float32))
    wb = np.ascontiguousarray(np.asarray(wb, dtype=np.float32))
    wt = np.ascontiguousarray(np.asarray(wt, dtype=np.float32))
    blob = _make_blob(wt)

    in_maps = []
    for c in range(N_CORES):
        in_maps.append({
            "xs": np.ascontiguousarray(x[:, c * BSH:(c + 1) * BSH]),
            "wa": wa, "wb": wb, "blob": blob,
        })
    res = bass_utils.run_bass_kernel_spmd(nc, in_maps, core_ids=list(range(N_CORES)),
                                          trace=trace, **spmd_kwargs)
    out = np.concatenate([res.results[c]["out"] for c in range(N_CORES)], axis=1)
    return out, res


def kernel(x, wa, wb, wt):
    out, _ = _run(x, wa, wb, wt, trace=False)
    return out



# revision 3
# speedup vs baseline: 1.0475x; 1.0475x over previous
"""Trainium2 Bass kernel for the fused soft-logic-gate layer.

Reference computation:
    pa = softmax(wa, axis=1); pb = softmax(wb, axis=1); pt = softmax(wt, axis=0)
    A = pa @ x; B = pb @ x
    out = sum_g pt[g,:,None] * gate_g(A, B)        (16 soft logic gates)

Every gate is affine in {1, A, B, A*B}, so the 16-gate table collapses to
    out = c0 + cA*A + cB*B + cAB*(A*B)
with per-row coefficients derived from pt.  All weight-only math (exp,
transposes, softmax denominators, the coefficient chain) is precomputed on
the host in float64; the device receives exp(wa)^T / exp(wb)^T (float32r)
plus five per-row coefficient vectors and computes, per batch tile:
    PSUM:  A = ea^T @ x ; B = eb^T @ x          (TensorE, f32r)
    ACT:   s = ss*B + ca                        (scalar engine)
    DVE:   p = (A + u) * s                      (scalar_tensor_tensor)
    o = ro*p + w  -> bf16                       (DVE for m=0, ACT for m=1)
using the numerically-safe factoring  out = (A_t+U)(cAB_t*B_t+cA_t)+W  with
the normalizations folded in (ss=cAB_t/rsb, ca=cA_t, u=U*rsa, ro=1/rsa,
w=W).  The output is stored bf16 (tolerance 2e-2; bf16 adds ~3e-3), halving
write traffic, and upcast to f32 on the host.

Sharding: batch axis of x split evenly across 8 NeuronCores (data parallel),
weights replicated.
"""

import os
import sys

for _p in ("/opt/trn_rl_repo",):
    if _p not in sys.path and os.path.isdir(_p):
        sys.path.insert(0, _p)

import numpy as np
import ml_dtypes

SIZE = 256
PREV = 256
BATCH = 32768
N_CORES = 8
BSH = BATCH // N_CORES  # per-core batch shard
CH = 1024               # chunk width (2 PSUM banks per mat)
NCH = BSH // CH
P = 128

_CACHE = {}


def _sign_matrix() -> np.ndarray:
    """[16,5] f64 columns: [colsum, c0, cA, cB, cAB] — gate-table
    coefficients of {1, A, B, A*B} preceded by the softmax denominator."""
    S = np.zeros((16, 5), dtype=np.float64)
    S[:, 0] = 1.0
    S[8:16, 1] = 1.0
    for g in (2, 3, 6, 7):
        S[g, 2] += 1.0
    for g in (8, 9, 12, 13):
        S[g, 2] -= 1.0
    for g in (4, 5, 6, 7):
        S[g, 3] += 1.0
    for g in (8, 9, 10, 11):
        S[g, 3] -= 1.0
    for g, v in {1: 1, 2: -1, 4: -1, 6: -2, 7: -1, 8: 1, 9: 2, 11: 1, 13: 1, 14: -1}.items():
        S[g, 4] = v
    return S


def _host_prep(wa, wb, wt):
    """f64 weight-only preprocessing -> (eat, ebt, coef) f32 arrays."""
    wa = wa.astype(np.float64)
    wb = wb.astype(np.float64)
    wt = wt.astype(np.float64)
    ea = np.exp(wa)                      # [size, prev]
    eb = np.exp(wb)
    rsa = ea.sum(axis=1)                 # [size]
    rsb = eb.sum(axis=1)
    cps = np.exp(wt).T @ _sign_matrix()  # [size, 5]
    Ssum, c0n, cAn, cBn, cABn = cps.T
    # normalized gate coefficients
    c0 = c0n / Ssum
    cA = cAn / Ssum
    cB = cBn / Ssum
    cAB = cABn / Ssum
    U = cB / cAB
    W = c0 - cA * U
    # device-side scalars (per out-row): s = ss*B + ca ; p = (A+u)*s ;
    # o = ro*p + w
    ss = cAB / rsb
    ca = cA
    u = U * rsa
    ro = 1.0 / rsa
    w = W
    coef = np.stack([u, ss, ca, ro, w], axis=0)          # [5, 256]
    coef = coef.reshape(5, 2, P).transpose(2, 0, 1)      # [128, 5, 2]
    coef = np.ascontiguousarray(coef.reshape(P, 10), dtype=np.float32)
    eat = np.ascontiguousarray(ea.T, dtype=np.float32)   # [prev, size]
    ebt = np.ascontiguousarray(eb.T, dtype=np.float32)
    return eat, ebt, coef


def _build_bass():
    import concourse.bacc as bacc
    import concourse.tile as tile
    import concourse.mybir as mybir

    f32 = mybir.dt.float32
    f32r = mybir.dt.float32r
    bf16 = mybir.dt.bfloat16
    Act = mybir.ActivationFunctionType
    Alu = mybir.AluOpType

    nc = bacc.Bacc(trn_type="TRN2", target_bir_lowering=False, debug=False,
                   num_devices=N_CORES)

    xs_d = nc.dram_tensor("xs", [PREV, BSH], f32r, kind="ExternalInput").ap()
    eat_d = nc.dram_tensor("eat", [PREV, SIZE], f32r, kind="ExternalInput").ap()
    ebt_d = nc.dram_tensor("ebt", [PREV, SIZE], f32r, kind="ExternalInput").ap()
    coef_d = nc.dram_tensor("coef", [P, 10], f32, kind="ExternalInput").ap()
    out_d = nc.dram_tensor("out", [SIZE, BSH], bf16, kind="ExternalOutput").ap()

    # [p, k, *] views for single-DMA transfers
    xs_v = xs_d.rearrange("(k p) b -> p k b", p=P)
    eat_v = eat_d.rearrange("(k p) m -> p k m", p=P)
    ebt_v = ebt_d.rearrange("(k p) m -> p k m", p=P)

    with tile.TileContext(nc) as tc:
        with tc.tile_pool(name="consts", bufs=1) as consts, \
             tc.tile_pool(name="xp", bufs=4) as xp, \
             tc.tile_pool(name="ep", bufs=3) as ep, \
             tc.tile_pool(name="mm_ps", bufs=2, space="PSUM") as mm_ps:

            # ACT table preload off the critical path (Identity set)
            dummy = consts.tile([1, 1], f32)
            nc.gpsimd.memset(dummy[:], 0.0)
            nc.scalar.activation(out=dummy[:], in_=dummy[:], func=Act.Identity,
                                 scale=1.0, bias=0.0)

            # input DMAs in dependency order on the SP (hwdge) queue
            ebT = consts.tile([P, 2, SIZE], f32r, tag="ebT")
            nc.sync.dma_start(out=ebT[:], in_=ebt_v[:])
            eaT = consts.tile([P, 2, SIZE], f32r, tag="eaT")
            nc.sync.dma_start(out=eaT[:], in_=eat_v[:])
            coef = consts.tile([P, 10], f32, tag="coef")
            nc.sync.dma_start(out=coef[:], in_=coef_d[:])
            cv = coef[:].rearrange("p (c m) -> p c m", c=5)
            u2, ss2, ca2, ro2, w2 = (cv[:, i, :] for i in range(5))

            xtiles = []
            for n in range(NCH):
                xt = xp.tile([P, 2, CH], f32r, tag="x", name=f"x{n}")
                nc.sync.dma_start(out=xt[:], in_=xs_v[:, :, n * CH:(n + 1) * CH])
                xtiles.append(xt)

            # ---- main loop ----
            for n in range(NCH):
                xk = xtiles[n]
                for m in range(2):
                    b_ps = mm_ps.tile([P, CH], f32, tag="B", name=f"B{n}{m}")
                    a_ps = mm_ps.tile([P, CH], f32, tag="A", name=f"A{n}{m}")
                    # B first: s-ACT consumes it while A's matmuls run
                    for k in range(2):
                        for s in range(CH // 512):
                            sl = slice(s * 512, (s + 1) * 512)
                            nc.tensor.matmul(b_ps[:, sl],
                                             ebT[:, k, m * P:(m + 1) * P],
                                             xk[:, k, sl],
                                             start=(k == 0), stop=(k == 1))
                    s_sb = ep.tile([P, CH], f32, tag="s", name=f"s{n}{m}")
                    nc.scalar.activation(out=s_sb[:], in_=b_ps[:], func=Act.Identity,
                                         scale=ss2[:, m:m + 1], bias=ca2[:, m:m + 1])
                    for k in range(2):
                        for s in range(CH // 512):
                            sl = slice(s * 512, (s + 1) * 512)
                            nc.tensor.matmul(a_ps[:, sl],
                                             eaT[:, k, m * P:(m + 1) * P],
                                             xk[:, k, sl],
                                             start=(k == 0), stop=(k == 1))
                    p_sb = ep.tile([P, CH], f32, tag="p", name=f"p{n}{m}")
                    nc.vector.scalar_tensor_tensor(out=p_sb[:], in0=a_ps[:],
                                                   scalar=u2[:, m:m + 1], in1=s_sb[:],
                                                   op0=Alu.add, op1=Alu.mult)
                    o_sb = ep.tile([P, CH], bf16, tag="o", name=f"o{n}{m}")
                    if m == 0:
                        nc.vector.tensor_scalar(out=o_sb[:], in0=p_sb[:],
                                                scalar1=ro2[:, m:m + 1],
                                                scalar2=w2[:, m:m + 1],
                                                op0=Alu.mult, op1=Alu.add)
                    else:
                        nc.scalar.activation(out=o_sb[:], in_=p_sb[:], func=Act.Identity,
                                             scale=ro2[:, m:m + 1], bias=w2[:, m:m + 1])
                    # out DMA on the gpsimd SWDGE queue (keeps the SP hwdge
                    # queue free for input loads)
                    nc.gpsimd.dma_start(out=out_d[m * P:(m + 1) * P, n * CH:(n + 1) * CH],
                                        in_=o_sb[:])

    nc.compile()
    return nc


def _get_nc():
    if "nc" not in _CACHE:
        _CACHE["nc"] = _build_bass()
    return _CACHE["nc"]


def _run(x, wa, wb, wt, trace=False, **spmd_kwargs):
    from concourse import bass_utils

    nc = _get_nc()
    x = np.ascontiguousarray(np.asarray(x, dtype=np.float32))
    wa = np.asarray(wa, dtype=np.float32)
    wb = np.asarray(wb, dtype=np.float32)
    wt = np.asarray(wt, dtype=np.float32)
    eat, ebt, coef = _host_prep(wa, wb, wt)

    in_maps = []
    for c in range(N_CORES):
        in_maps.append({
            "xs": np.ascontiguousarray(x[:, c * BSH:(c + 1) * BSH]),
            "eat": eat, "ebt": ebt, "coef": coef,
        })
    res = bass_utils.run_bass_kernel_spmd(nc, in_maps, core_ids=list(range(N_CORES)),
                                          trace=trace, **spmd_kwargs)
    out = np.concatenate(
        [res.results[c]["out"].astype(np.float32) for c in range(N_CORES)], axis=1)
    return out, res


def kernel(x, wa, wb, wt):
    out, _ = _run(x, wa, wb, wt, trace=False)
    return out


# revision 4
# speedup vs baseline: 1.0831x; 1.0340x over previous
"""Trainium2 Bass kernel for the fused soft-logic-gate layer.

Reference computation:
    pa = softmax(wa, axis=1); pb = softmax(wb, axis=1); pt = softmax(wt, axis=0)
    A = pa @ x; B = pb @ x
    out = sum_g pt[g,:,None] * gate_g(A, B)        (16 soft logic gates)

Every gate is affine in {1, A, B, A*B}, so the 16-gate table collapses to
    out = c0 + cA*A + cB*B + cAB*(A*B)
with per-row coefficients derived from pt.  All weight-only math (exp,
transposes, softmax denominators, the coefficient chain) is precomputed on
the host in float64; the device receives exp(wa)^T / exp(wb)^T (float32r)
plus five per-row coefficient vectors and computes, per batch tile:
    PSUM:  A = ea^T @ x ; B = eb^T @ x          (TensorE, f32r)
    ACT:   s = ss*B + ca                        (scalar engine)
    DVE:   p = (A + u) * s                      (scalar_tensor_tensor)
    o = ro*p + w  -> bf16                       (DVE for m=0, ACT for m=1)
using the numerically-safe factoring  out = (A_t+U)(cAB_t*B_t+cA_t)+W  with
the normalizations folded in (ss=cAB_t/rsb, ca=cA_t, u=U*rsa, ro=1/rsa,
w=W).  The output is stored bf16 (tolerance 2e-2; bf16 adds ~3e-3), halving
write traffic, and upcast to f32 on the host.

Schedule notes: all DMAs ride the SP hardware-DGE queue (inputs enqueued
first, so the in-order queue never blocks them behind outputs); the first x
chunk is split in two so matmuls start earlier; B's matmuls precede A's so
the scalar-engine `s` op overlaps A's matmuls and PSUM banks recycle without
stalling the PE; a burst of throwaway warm-up matmuls ramps the PE out of
its low-clock p-state before the real stream arrives.

Sharding: batch axis of x split evenly across 8 NeuronCores (data parallel),
weights replicated.
"""

import os
import sys

for _p in ("/opt/trn_rl_repo",):
    if _p not in sys.path and os.path.isdir(_p):
        sys.path.insert(0, _p)

import numpy as np
import ml_dtypes

SIZE = 256
PREV = 256
BATCH = 32768
N_CORES = 8
BSH = BATCH // N_CORES  # per-core batch shard
CH = 1024               # chunk width (2 PSUM banks per mat)
NCH = BSH // CH
P = 128
NWARM = 10              # PE p-state warm-up matmuls

_CACHE = {}


def _sign_matrix() -> np.ndarray:
    """[16,5] f64 columns: [colsum, c0, cA, cB, cAB] — gate-table
    coefficients of {1, A, B, A*B} preceded by the softmax denominator."""
    S = np.zeros((16, 5), dtype=np.float64)
    S[:, 0] = 1.0
    S[8:16, 1] = 1.0
    for g in (2, 3, 6, 7):
        S[g, 2] += 1.0
    for g in (8, 9, 12, 13):
        S[g, 2] -= 1.0
    for g in (4, 5, 6, 7):
        S[g, 3] += 1.0
    for g in (8, 9, 10, 11):
        S[g, 3] -= 1.0
    for g, v in {1: 1, 2: -1, 4: -1, 6: -2, 7: -1, 8: 1, 9: 2, 11: 1, 13: 1, 14: -1}.items():
        S[g, 4] = v
    return S


def _host_prep(wa, wb, wt):
    """f64 weight-only preprocessing -> (eat, ebt, coef) f32 arrays."""
    wa = wa.astype(np.float64)
    wb = wb.astype(np.float64)
    wt = wt.astype(np.float64)
    ea = np.exp(wa)                      # [size, prev]
    eb = np.exp(wb)
    rsa = ea.sum(axis=1)                 # [size]
    rsb = eb.sum(axis=1)
    cps = np.exp(wt).T @ _sign_matrix()  # [size, 5]
    Ssum, c0n, cAn, cBn, cABn = cps.T
    # normalized gate coefficients
    c0 = c0n / Ssum
    cA = cAn / Ssum
    cB = cBn / Ssum
    cAB = cABn / Ssum
    U = cB / cAB
    W = c0 - cA * U
    # device-side per-row scalars: s = ss*B + ca ; p = (A+u)*s ; o = ro*p + w
    ss = cAB / rsb
    ca = cA
    u = U * rsa
    ro = 1.0 / rsa
    w = W
    coef = np.stack([u, ss, ca, ro, w], axis=0)          # [5, 256]
    coef = coef.reshape(5, 2, P).transpose(2, 0, 1)      # [128, 5, 2]
    coef = np.ascontiguousarray(coef.reshape(P, 10), dtype=np.float32)
    eat = np.ascontiguousarray(ea.T, dtype=np.float32)   # [prev, size]
    ebt = np.ascontiguousarray(eb.T, dtype=np.float32)
    return eat, ebt, coef


def _build_bass():
    import concourse.bacc as bacc
    import concourse.tile as tile
    import concourse.mybir as mybir

    f32 = mybir.dt.float32
    f32r = mybir.dt.float32r
    bf16 = mybir.dt.bfloat16
    Act = mybir.ActivationFunctionType
    Alu = mybir.AluOpType

    nc = bacc.Bacc(trn_type="TRN2", target_bir_lowering=False, debug=False,
                   num_devices=N_CORES)

    xs_d = nc.dram_tensor("xs", [PREV, BSH], f32r, kind="ExternalInput").ap()
    eat_d = nc.dram_tensor("eat", [PREV, SIZE], f32r, kind="ExternalInput").ap()
    ebt_d = nc.dram_tensor("ebt", [PREV, SIZE], f32r, kind="ExternalInput").ap()
    coef_d = nc.dram_tensor("coef", [P, 10], f32, kind="ExternalInput").ap()
    out_d = nc.dram_tensor("out", [SIZE, BSH], bf16, kind="ExternalOutput").ap()

    # [p, k/m, *] views for single-DMA transfers
    xs_v = xs_d.rearrange("(k p) b -> p k b", p=P)
    eat_v = eat_d.rearrange("(k p) m -> p k m", p=P)
    ebt_v = ebt_d.rearrange("(k p) m -> p k m", p=P)
    out_v = out_d.rearrange("(m p) b -> p m b", p=P)

    with tile.TileContext(nc) as tc:
        with tc.tile_pool(name="consts", bufs=1) as consts, \
             tc.tile_pool(name="xp", bufs=1) as xp, \
             tc.tile_pool(name="ep", bufs=3) as ep:

            # input DMAs in dependency order on the SP (hwdge) queue;
            # first x chunk split in two so matmuls can start sooner
            ebT = consts.tile([P, 2, SIZE], f32r, tag="ebT")
            nc.sync.dma_start(out=ebT[:], in_=ebt_v[:])
            x0 = [xp.tile([P, 2, 512], f32r, tag=f"x0{h}", name=f"x0{h}")
                  for h in range(2)]
            nc.sync.dma_start(out=x0[0][:], in_=xs_v[:, :, 0:512])
            eaT = consts.tile([P, 2, SIZE], f32r, tag="eaT")
            nc.sync.dma_start(out=eaT[:], in_=eat_v[:])
            nc.sync.dma_start(out=x0[1][:], in_=xs_v[:, :, 512:1024])
            coef = consts.tile([P, 10], f32, tag="coef")
            nc.sync.dma_start(out=coef[:], in_=coef_d[:])
            cv = coef[:].rearrange("p (c m) -> p c m", c=5)
            u2, ss2, ca2, ro2, w2 = (cv[:, i, :] for i in range(5))

            xtiles = [None]
            for n in range(1, NCH):
                xt = xp.tile([P, 2, CH], f32r, tag=f"x{n}", name=f"x{n}")
                nc.sync.dma_start(out=xt[:], in_=xs_v[:, :, n * CH:(n + 1) * CH])
                xtiles.append(xt)

            # PE p-state warm-up: throwaway matmuls on ebT, result never read
            with tc.tile_pool(name="warm_ps", bufs=1, space="PSUM") as warm_ps:
                wps = warm_ps.tile([P, 256], f32, tag="wps")
                for r in range(NWARM):
                    nc.tensor.matmul(wps[:], ebT[:, 0, 0:P], ebT[:, 0, 0:256],
                                     start=True, stop=True, skip_group_check=True)

            # ---- main loop ----
            with tc.tile_pool(name="mm_ps", bufs=2, space="PSUM") as mm_ps:
                for n in range(NCH):
                    def rhs(k, s):
                        if n == 0:
                            return x0[s][:, k, :]
                        return xtiles[n][:, k, s * 512:(s + 1) * 512]

                    o_sb = ep.tile([P, 2, CH], bf16, tag="o", name=f"o{n}")
                    for m in range(2):
                        b_ps = mm_ps.tile([P, CH], f32, tag="B", name=f"B{n}{m}")
                        a_ps = mm_ps.tile([P, CH], f32, tag="A", name=f"A{n}{m}")
                        # B first: the s-ACT consumes it while A's matmuls run
                        for s in range(CH // 512):
                            sl = slice(s * 512, (s + 1) * 512)
                            for k in range(2):
                                nc.tensor.matmul(b_ps[:, sl],
                                                 ebT[:, k, m * P:(m + 1) * P],
                                                 rhs(k, s),
                                                 start=(k == 0), stop=(k == 1))
                        s_sb = ep.tile([P, CH], f32, tag="s", name=f"s{n}{m}")
                        nc.scalar.activation(out=s_sb[:], in_=b_ps[:], func=Act.Identity,
                                             scale=ss2[:, m:m + 1], bias=ca2[:, m:m + 1])
                        for s in range(CH // 512):
                            sl = slice(s * 512, (s + 1) * 512)
                            for k in range(2):
                                nc.tensor.matmul(a_ps[:, sl],
                                                 eaT[:, k, m * P:(m + 1) * P],
                                                 rhs(k, s),
                                                 start=(k == 0), stop=(k == 1))
                        p_sb = ep.tile([P, CH], f32, tag="p", name=f"p{n}{m}")
                        nc.vector.scalar_tensor_tensor(out=p_sb[:], in0=a_ps[:],
                                                       scalar=u2[:, m:m + 1], in1=s_sb[:],
                                                       op0=Alu.add, op1=Alu.mult)
                        if m == 0:
                            nc.vector.tensor_scalar(out=o_sb[:, 0, :], in0=p_sb[:],
                                                    scalar1=ro2[:, m:m + 1],
                                                    scalar2=w2[:, m:m + 1],
                                                    op0=Alu.mult, op1=Alu.add)
                        else:
                            nc.scalar.activation(out=o_sb[:, 1, :], in_=p_sb[:],
                                                 func=Act.Identity,
                                                 scale=ro2[:, m:m + 1], bias=w2[:, m:m + 1])
                    nc.sync.dma_start(out=out_v[:, :, n * CH:(n + 1) * CH], in_=o_sb[:])

    nc.compile()
    return nc


def _get_nc():
    if "nc" not in _CACHE:
        _CACHE["nc"] = _build_bass()
    return _CACHE["nc"]


def _run(x, wa, wb, wt, trace=False, **spmd_kwargs):
    from concourse import bass_utils

    nc = _get_nc()
    x = np.ascontiguousarray(np.asarray(x, dtype=np.float32))
    wa = np.asarray(wa, dtype=np.float32)
    wb = np.asarray(wb, dtype=np.float32)
    wt = np.asarray(wt, dtype=np.float32)
    eat, ebt, coef = _host_prep(wa, wb, wt)

    in_maps = []
    for c in range(N_CORES):
        in_maps.append({
            "xs": np.ascontiguousarray(x[:, c * BSH:(c + 1) * BSH]),
            "eat": eat, "ebt": ebt, "coef": coef,
        })
    res = bass_utils.run_bass_kernel_spmd(nc, in_maps, core_ids=list(range(N_CORES)),
                                          trace=trace, **spmd_kwargs)
    out = np.concatenate(
        [res.results[c]["out"].astype(np.float32) for c in range(N_CORES)], axis=1)
    return out, res


def kernel(x, wa, wb, wt):
    out, _ = _run(x, wa, wb, wt, trace=False)
    return out


# revision 6
# speedup vs baseline: 1.1445x; 1.0567x over previous
"""Trainium2 Bass kernel for the fused soft-logic-gate layer.

Reference computation:
    pa = softmax(wa, axis=1); pb = softmax(wb, axis=1); pt = softmax(wt, axis=0)
    A = pa @ x; B = pb @ x
    out = sum_g pt[g,:,None] * gate_g(A, B)        (16 soft logic gates)

Every gate is affine in {1, A, B, A*B}, so the 16-gate table collapses to
    out = c0 + cA*A + cB*B + cAB*(A*B)
with per-row coefficients derived from pt.  All weight-only math (exp,
transposes, softmax denominators, the coefficient chain) is precomputed on
the host in float64; the device receives exp(wa)^T / exp(wb)^T (float32r)
plus five per-row coefficient vectors and computes, per batch tile:
    PSUM:  A = ea^T @ x ; B = eb^T @ x          (TensorE, f32r)
    ACT:   s = ss*B + ca                        (scalar engine)
    DVE:   p = (A + u) * s                      (scalar_tensor_tensor)
    o = ro*p + w  -> bf16                       (DVE for m=0, ACT for m=1)
using the numerically-safe factoring  out = (A_t+U)(cAB_t*B_t+cA_t)+W  with
the normalizations folded in (ss=cAB_t/rsb, ca=cA_t, u=U*rsa, ro=1/rsa,
w=W).  The output is stored bf16 (tolerance 2e-2; bf16 adds ~3e-3), halving
write traffic, and upcast to f32 on the host.

Schedule notes: all DMAs ride the SP hardware-DGE queue (inputs enqueued
first, so the in-order queue never blocks them behind outputs); x arrives in
512-column tiles so the final chunk's matmuls start right after the input
stream ends; B's matmuls precede A's so the scalar-engine `s` op overlaps
A's matmuls and PSUM banks recycle without stalling the PE; a burst of
throwaway matmuls on scratch SBUF ramps the PE out of its low-clock p-state
during the DMA head; the last chunk's epilogue runs in half-tiles to
shorten the drain.

Sharding: batch axis of x split evenly across 8 NeuronCores (data parallel),
weights replicated.
"""

import os
import sys

for _p in ("/opt/trn_rl_repo",):
    if _p not in sys.path and os.path.isdir(_p):
        sys.path.insert(0, _p)

import numpy as np
import ml_dtypes

SIZE = 256
PREV = 256
BATCH = 32768
N_CORES = 8
BSH = BATCH // N_CORES  # per-core batch shard
CH = 1024               # chunk width (2 PSUM banks per mat)
NCH = BSH // CH
P = 128
NWARM = 12              # PE p-state warm-up matmuls

_CACHE = {}


def _sign_matrix() -> np.ndarray:
    """[16,5] f64 columns: [colsum, c0, cA, cB, cAB] — gate-table
    coefficients of {1, A, B, A*B} preceded by the softmax denominator."""
    S = np.zeros((16, 5), dtype=np.float64)
    S[:, 0] = 1.0
    S[8:16, 1] = 1.0
    for g in (2, 3, 6, 7):
        S[g, 2] += 1.0
    for g in (8, 9, 12, 13):
        S[g, 2] -= 1.0
    for g in (4, 5, 6, 7):
        S[g, 3] += 1.0
    for g in (8, 9, 10, 11):
        S[g, 3] -= 1.0
    for g, v in {1: 1, 2: -1, 4: -1, 6: -2, 7: -1, 8: 1, 9: 2, 11: 1, 13: 1, 14: -1}.items():
        S[g, 4] = v
    return S


def _host_prep(wa, wb, wt):
    """f64 weight-only preprocessing -> (eat, ebt, coef) f32 arrays."""
    wa = wa.astype(np.float64)
    wb = wb.astype(np.float64)
    wt = wt.astype(np.float64)
    ea = np.exp(wa)                      # [size, prev]
    eb = np.exp(wb)
    rsa = ea.sum(axis=1)                 # [size]
    rsb = eb.sum(axis=1)
    cps = np.exp(wt).T @ _sign_matrix()  # [size, 5]
    Ssum, c0n, cAn, cBn, cABn = cps.T
    # normalized gate coefficients
    c0 = c0n / Ssum
    cA = cAn / Ssum
    cB = cBn / Ssum
    cAB = cABn / Ssum
    U = cB / cAB
    W = c0 - cA * U
    # device-side per-row scalars: s = ss*B + ca ; p = (A+u)*s ; o = ro*p + w
    ss = cAB / rsb
    ca = cA
    u = U * rsa
    ro = 1.0 / rsa
    w = W
    coef = np.stack([u, ss, ca, ro, w], axis=0)          # [5, 256]
    coef = coef.reshape(5, 2, P).transpose(2, 0, 1)      # [128, 5, 2]
    coef = np.ascontiguousarray(coef.reshape(P, 10), dtype=np.float32)
    eat = np.ascontiguousarray(ea.T, dtype=np.float32)   # [prev, size]
    ebt = np.ascontiguousarray(eb.T, dtype=np.float32)
    return eat, ebt, coef


def _build_bass():
    import concourse.bacc as bacc
    import concourse.tile as tile
    import concourse.mybir as mybir

    f32 = mybir.dt.float32
    f32r = mybir.dt.float32r
    bf16 = mybir.dt.bfloat16
    Act = mybir.ActivationFunctionType
    Alu = mybir.AluOpType

    nc = bacc.Bacc(trn_type="TRN2", target_bir_lowering=False, debug=False,
                   num_devices=N_CORES)

    xs_d = nc.dram_tensor("xs", [PREV, BSH], f32r, kind="ExternalInput").ap()
    eat_d = nc.dram_tensor("eat", [PREV, SIZE], f32r, kind="ExternalInput").ap()
    ebt_d = nc.dram_tensor("ebt", [PREV, SIZE], f32r, kind="ExternalInput").ap()
    coef_d = nc.dram_tensor("coef", [P, 10], f32, kind="ExternalInput").ap()
    out_d = nc.dram_tensor("out", [SIZE, BSH], bf16, kind="ExternalOutput").ap()

    # [p, k, *] views for single-DMA transfers
    xs_v = xs_d.rearrange("(k p) b -> p k b", p=P)
    eat_v = eat_d.rearrange("(k p) m -> p k m", p=P)
    ebt_v = ebt_d.rearrange("(k p) m -> p k m", p=P)

    NS = BSH // 512  # number of 512-wide x tiles

    with tile.TileContext(nc) as tc:
        with tc.tile_pool(name="consts", bufs=1) as consts, \
             tc.tile_pool(name="xp", bufs=1) as xp, \
             tc.tile_pool(name="ep", bufs=3) as ep:

            # PE p-state warm-up on scratch SBUF (values irrelevant)
            scratch = consts.tile([P, 256], f32, tag="scratch")
            nc.gpsimd.memset(scratch[:], 1.0)

            # input DMAs in dependency order on the SP (hwdge) queue
            ebT = consts.tile([P, 2, SIZE], f32r, tag="ebT")
            nc.sync.dma_start(out=ebT[:], in_=ebt_v[:])
            xtiles = [xp.tile([P, 2, 512], f32r, tag=f"x{t}", name=f"x{t}")
                      for t in range(NS)]
            nc.sync.dma_start(out=xtiles[0][:], in_=xs_v[:, :, 0:512])
            coef = consts.tile([P, 10], f32, tag="coef")
            nc.sync.dma_start(out=coef[:], in_=coef_d[:])
            eaT = consts.tile([P, 2, SIZE], f32r, tag="eaT")
            nc.sync.dma_start(out=eaT[:], in_=eat_v[:])
            for t in range(1, NS):
                nc.sync.dma_start(out=xtiles[t][:],
                                  in_=xs_v[:, :, t * 512:(t + 1) * 512])

            cv = coef[:].rearrange("p (c m) -> p c m", c=5)
            u2, ss2, ca2, ro2, w2 = (cv[:, i, :] for i in range(5))

            with tc.tile_pool(name="warm_ps", bufs=1, space="PSUM") as warm_ps:
                wps = warm_ps.tile([P, 256], f32, tag="wps")
                for r in range(NWARM):
                    nc.tensor.matmul(wps[:], scratch[:, 0:P], scratch[:],
                                     start=True, stop=True, skip_group_check=True)

            # ---- main loop ----
            with tc.tile_pool(name="mm_ps", bufs=2, space="PSUM") as mm_ps:
                for n in range(NCH):
                    last = n == NCH - 1
                    for m in range(2):
                        b_ps = mm_ps.tile([P, CH], f32, tag="B", name=f"B{n}{m}")
                        a_ps = mm_ps.tile([P, CH], f32, tag="A", name=f"A{n}{m}")
                        # B first: the s-ACT consumes it while A's matmuls run
                        for s in range(CH // 512):
                            sl = slice(s * 512, (s + 1) * 512)
                            xt = xtiles[2 * n + s]
                            for k in range(2):
                                nc.tensor.matmul(b_ps[:, sl],
                                                 ebT[:, k, m * P:(m + 1) * P],
                                                 xt[:, k, :],
                                                 start=(k == 0), stop=(k == 1))
                        s_sb = ep.tile([P, CH], f32, tag="s", name=f"s{n}{m}")
                        if last:
                            for h in range(2):
                                hl = slice(h * 512, (h + 1) * 512)
                                nc.scalar.activation(out=s_sb[:, hl], in_=b_ps[:, hl],
                                                     func=Act.Identity,
                                                     scale=ss2[:, m:m + 1],
                                                     bias=ca2[:, m:m + 1])
                        else:
                            nc.scalar.activation(out=s_sb[:], in_=b_ps[:],
                                                 func=Act.Identity,
                                                 scale=ss2[:, m:m + 1],
                                                 bias=ca2[:, m:m + 1])
                        for s in range(CH // 512):
                            sl = slice(s * 512, (s + 1) * 512)
                            xt = xtiles[2 * n + s]
                            for k in range(2):
                                nc.tensor.matmul(a_ps[:, sl],
                                                 eaT[:, k, m * P:(m + 1) * P],
                                                 xt[:, k, :],
                                                 start=(k == 0), stop=(k == 1))
                        p_sb = ep.tile([P, CH], f32, tag="p", name=f"p{n}{m}")
                        o_sb = ep.tile([P, CH], bf16, tag="o", name=f"o{n}{m}")
                        halves = (slice(0, 512), slice(512, CH)) if last else (slice(0, CH),)
                        for hl in halves:
                            nc.vector.scalar_tensor_tensor(out=p_sb[:, hl], in0=a_ps[:, hl],
                                                           scalar=u2[:, m:m + 1],
                                                           in1=s_sb[:, hl],
                                                           op0=Alu.add, op1=Alu.mult)
                            if m == 0:
                                nc.vector.tensor_scalar(out=o_sb[:, hl], in0=p_sb[:, hl],
                                                        scalar1=ro2[:, m:m + 1],
                                                        scalar2=w2[:, m:m + 1],
                                                        op0=Alu.mult, op1=Alu.add)
                            else:
                                nc.scalar.activation(out=o_sb[:, hl], in_=p_sb[:, hl],
                                                     func=Act.Identity,
                                                     scale=ro2[:, m:m + 1],
                                                     bias=w2[:, m:m + 1])
                            nc.sync.dma_start(
                                out=out_d[m * P:(m + 1) * P,
                                          n * CH + hl.start:n * CH + hl.stop],
                                in_=o_sb[:, hl])

    nc.compile()
    return nc


def _get_nc():
    if "nc" not in _CACHE:
        _CACHE["nc"] = _build_bass()
    return _CACHE["nc"]


def _run(x, wa, wb, wt, trace=False, **spmd_kwargs):
    from concourse import bass_utils

    nc = _get_nc()
    x = np.ascontiguousarray(np.asarray(x, dtype=np.float32))
    wa = np.asarray(wa, dtype=np.float32)
    wb = np.asarray(wb, dtype=np.float32)
    wt = np.asarray(wt, dtype=np.float32)
    eat, ebt, coef = _host_prep(wa, wb, wt)

    in_maps = []
    for c in range(N_CORES):
        in_maps.append({
            "xs": np.ascontiguousarray(x[:, c * BSH:(c + 1) * BSH]),
            "eat": eat, "ebt": ebt, "coef": coef,
        })
    res = bass_utils.run_bass_kernel_spmd(nc, in_maps, core_ids=list(range(N_CORES)),
                                          trace=trace, **spmd_kwargs)
    out = np.concatenate(
        [res.results[c]["out"].astype(np.float32) for c in range(N_CORES)], axis=1)
    return out, res


def kernel(x, wa, wb, wt):
    out, _ = _run(x, wa, wb, wt, trace=False)
    return out


# revision 7
# speedup vs baseline: 1.1947x; 1.0438x over previous
"""Trainium2 Bass kernel for the fused soft-logic-gate layer.

Reference computation:
    pa = softmax(wa, axis=1); pb = softmax(wb, axis=1); pt = softmax(wt, axis=0)
    A = pa @ x; B = pb @ x
    out = sum_g pt[g,:,None] * gate_g(A, B)        (16 soft logic gates)

Every gate is affine in {1, A, B, A*B}, so the 16-gate table collapses to
    out = c0 + cA*A + cB*B + cAB*(A*B)
with per-row coefficients derived from pt.  All weight-only math (exp,
transposes, softmax denominators, the coefficient chain) is precomputed on
the host in float64; x and the exp-weights are cast to bf16 on the host
(tolerance is 2e-2; bf16 contributes ~3e-3 and halves both HBM traffic and
matmul time, and enables the PE fast-weight-load path).  The device
computes, per batch tile:
    PSUM:  A = ea^T @ x ; B = eb^T @ x          (TensorE, bf16 -> f32)
    ACT:   s = ss*B + ca                        (scalar engine)
    DVE:   p = (A + u) * s                      (scalar_tensor_tensor)
    o = ro*p + w  -> bf16                       (DVE for m=0, ACT for m=1)
using the numerically-safe factoring  out = (A_t+U)(cAB_t*B_t+cA_t)+W  with
the normalizations folded in (ss=cAB_t/rsb, ca=cA_t, u=U*rsa, ro=1/rsa,
w=W); the f32 PSUM/intermediates keep the near-singular-cAB rows exact.
Output is stored bf16 and upcast to f32 on the host.

Schedule notes: all DMAs ride the SP hardware-DGE queue, inputs enqueued
first; B's matmuls precede A's so the scalar-engine `s` op overlaps A's
matmuls and PSUM banks recycle without stalling the PE; a short burst of
throwaway matmuls on memset scratch ramps the PE out of its low-clock
p-state during the DMA head; the final tile's epilogue runs in half-tiles
to shorten the drain, and epilogue affine ops alternate DVE/ACT to balance
the two engines.

Sharding: batch axis of x split evenly across 8 NeuronCores (data parallel),
weights replicated.
"""

import os
import sys

for _p in ("/opt/trn_rl_repo",):
    if _p not in sys.path and os.path.isdir(_p):
        sys.path.insert(0, _p)

import numpy as np
import ml_dtypes

SIZE = 256
PREV = 256
BATCH = 32768
N_CORES = 8
BSH = BATCH // N_CORES  # per-core batch shard
CH = 1024               # chunk width (2 PSUM banks per mat)
NCH = BSH // CH
P = 128
NWARM = 6               # PE p-state warm-up matmuls

_CACHE = {}


def _sign_matrix() -> np.ndarray:
    """[16,5] f64 columns: [colsum, c0, cA, cB, cAB] — gate-table
    coefficients of {1, A, B, A*B} preceded by the softmax denominator."""
    S = np.zeros((16, 5), dtype=np.float64)
    S[:, 0] = 1.0
    S[8:16, 1] = 1.0
    for g in (2, 3, 6, 7):
        S[g, 2] += 1.0
    for g in (8, 9, 12, 13):
        S[g, 2] -= 1.0
    for g in (4, 5, 6, 7):
        S[g, 3] += 1.0
    for g in (8, 9, 10, 11):
        S[g, 3] -= 1.0
    for g, v in {1: 1, 2: -1, 4: -1, 6: -2, 7: -1, 8: 1, 9: 2, 11: 1, 13: 1, 14: -1}.items():
        S[g, 4] = v
    return S


def _host_prep(wa, wb, wt):
    """f64 weight-only preprocessing -> (eat, ebt, coef) device arrays."""
    wa = wa.astype(np.float64)
    wb = wb.astype(np.float64)
    wt = wt.astype(np.float64)
    ea = np.exp(wa)                      # [size, prev]
    eb = np.exp(wb)
    # the matmuls run on bf16-rounded weights; fold the matching row sums
    eat = ea.T.astype(ml_dtypes.bfloat16)   # [prev, size]
    ebt = eb.T.astype(ml_dtypes.bfloat16)
    rsa = eat.astype(np.float64).sum(axis=0)
    rsb = ebt.astype(np.float64).sum(axis=0)
    cps = np.exp(wt).T @ _sign_matrix()  # [size, 5]
    Ssum, c0n, cAn, cBn, cABn = cps.T
    # normalized gate coefficients
    c0 = c0n / Ssum
    cA = cAn / Ssum
    cB = cBn / Ssum
    cAB = cABn / Ssum
    U = cB / cAB
    W = c0 - cA * U
    # device-side per-row scalars: s = ss*B + ca ; p = (A+u)*s ; o = ro*p + w
    ss = cAB / rsb
    ca = cA
    u = U * rsa
    ro = 1.0 / rsa
    w = W
    coef = np.stack([u, ss, ca, ro, w], axis=0)          # [5, 256]
    coef = coef.reshape(5, 2, P).transpose(2, 0, 1)      # [128, 5, 2]
    coef = np.ascontiguousarray(coef.reshape(P, 10), dtype=np.float32)
    return np.ascontiguousarray(eat), np.ascontiguousarray(ebt), coef


def _build_bass():
    import concourse.bacc as bacc
    import concourse.tile as tile
    import concourse.mybir as mybir

    f32 = mybir.dt.float32
    bf16 = mybir.dt.bfloat16
    Act = mybir.ActivationFunctionType
    Alu = mybir.AluOpType

    nc = bacc.Bacc(trn_type="TRN2", target_bir_lowering=False, debug=False,
                   num_devices=N_CORES)

    xs_d = nc.dram_tensor("xs", [PREV, BSH], bf16, kind="ExternalInput").ap()
    eat_d = nc.dram_tensor("eat", [PREV, SIZE], bf16, kind="ExternalInput").ap()
    ebt_d = nc.dram_tensor("ebt", [PREV, SIZE], bf16, kind="ExternalInput").ap()
    coef_d = nc.dram_tensor("coef", [P, 10], f32, kind="ExternalInput").ap()
    out_d = nc.dram_tensor("out", [SIZE, BSH], bf16, kind="ExternalOutput").ap()

    # [p, k, *] views for single-DMA transfers
    xs_v = xs_d.rearrange("(k p) b -> p k b", p=P)
    eat_v = eat_d.rearrange("(k p) m -> p k m", p=P)
    ebt_v = ebt_d.rearrange("(k p) m -> p k m", p=P)

    with tile.TileContext(nc) as tc:
        with tc.tile_pool(name="consts", bufs=1) as consts, \
             tc.tile_pool(name="ep", bufs=3) as ep:

            # PE p-state warm-up on scratch SBUF (values irrelevant)
            scratch = consts.tile([P, 256], bf16, tag="scratch")
            nc.gpsimd.memset(scratch[:], 1.0)

            # input DMAs in dependency order on the SP (hwdge) queue;
            # last chunk's x split in two for a shorter drain
            ebT = consts.tile([P, 2, SIZE], bf16, tag="ebT")
            nc.sync.dma_start(out=ebT[:], in_=ebt_v[:])
            xtiles = [consts.tile([P, 2, CH], bf16, tag=f"x{t}", name=f"x{t}")
                      for t in range(NCH - 1)]
            xlast = [consts.tile([P, 2, 512], bf16, tag=f"xl{h}", name=f"xl{h}")
                     for h in range(2)]
            nc.sync.dma_start(out=xtiles[0][:], in_=xs_v[:, :, 0:CH])
            coef = consts.tile([P, 10], f32, tag="coef")
            nc.sync.dma_start(out=coef[:], in_=coef_d[:])
            eaT = consts.tile([P, 2, SIZE], bf16, tag="eaT")
            nc.sync.dma_start(out=eaT[:], in_=eat_v[:])
            for t in range(1, NCH - 1):
                nc.sync.dma_start(out=xtiles[t][:],
                                  in_=xs_v[:, :, t * CH:(t + 1) * CH])
            for h in range(2):
                base = (NCH - 1) * CH + h * 512
                nc.sync.dma_start(out=xlast[h][:],
                                  in_=xs_v[:, :, base:base + 512])

            cv = coef[:].rearrange("p (c m) -> p c m", c=5)
            u2, ss2, ca2, ro2, w2 = (cv[:, i, :] for i in range(5))

            with tc.tile_pool(name="warm_ps", bufs=1, space="PSUM") as warm_ps:
                wps = warm_ps.tile([P, 256], f32, tag="wps")
                for r in range(NWARM):
                    nc.tensor.matmul(wps[:], scratch[:, 0:P], scratch[:],
                                     start=True, stop=True, skip_group_check=True)

            def xslab(n, s):
                if n == NCH - 1:
                    return xlast[s]
                return xtiles[n][:, :, s * 512:(s + 1) * 512]

            # ---- main loop ----
            with tc.tile_pool(name="mm_ps", bufs=2, space="PSUM") as mm_ps:
                for n in range(NCH):
                    for m in range(2):
                        final = n == NCH - 1 and m == 1
                        b_ps = mm_ps.tile([P, CH], f32, tag="B", name=f"B{n}{m}")
                        a_ps = mm_ps.tile([P, CH], f32, tag="A", name=f"A{n}{m}")
                        # B first: the s-ACT consumes it while A's matmuls run
                        for s in range(CH // 512):
                            sl = slice(s * 512, (s + 1) * 512)
                            xt = xslab(n, s)
                            for k in range(2):
                                nc.tensor.matmul(b_ps[:, sl],
                                                 ebT[:, k, m * P:(m + 1) * P],
                                                 xt[:, k, :],
                                                 start=(k == 0), stop=(k == 1))
                        s_sb = ep.tile([P, CH], f32, tag="s", name=f"s{n}{m}")
                        s_halves = (slice(0, 512), slice(512, CH)) if final else (slice(0, CH),)
                        for hl in s_halves:
                            nc.scalar.activation(out=s_sb[:, hl], in_=b_ps[:, hl],
                                                 func=Act.Identity,
                                                 scale=ss2[:, m:m + 1],
                                                 bias=ca2[:, m:m + 1])
                        for s in range(CH // 512):
                            sl = slice(s * 512, (s + 1) * 512)
                            xt = xslab(n, s)
                            for k in range(2):
                                nc.tensor.matmul(a_ps[:, sl],
                                                 eaT[:, k, m * P:(m + 1) * P],
                                                 xt[:, k, :],
                                                 start=(k == 0), stop=(k == 1))
                        p_sb = ep.tile([P, CH], f32, tag="p", name=f"p{n}{m}")
                        o_sb = ep.tile([P, CH], bf16, tag="o", name=f"o{n}{m}")
                        for hl in s_halves:
                            nc.vector.scalar_tensor_tensor(out=p_sb[:, hl],
                                                           in0=a_ps[:, hl],
                                                           scalar=u2[:, m:m + 1],
                                                           in1=s_sb[:, hl],
                                                           op0=Alu.add, op1=Alu.mult)
                            # o = ro*p + w: DVE for m=0 and the final tile,
                            # ACT otherwise (engine balance)
                            if m == 0 or final:
                                nc.vector.tensor_scalar(out=o_sb[:, hl], in0=p_sb[:, hl],
                                                        scalar1=ro2[:, m:m + 1],
                                                        scalar2=w2[:, m:m + 1],
                                                        op0=Alu.mult, op1=Alu.add)
                            else:
                                nc.scalar.activation(out=o_sb[:, hl], in_=p_sb[:, hl],
                                                     func=Act.Identity,
                                                     scale=ro2[:, m:m + 1],
                                                     bias=w2[:, m:m + 1])
                            nc.sync.dma_start(
                                out=out_d[m * P:(m + 1) * P,
                                          n * CH + hl.start:n * CH + hl.stop],
                                in_=o_sb[:, hl])

    nc.compile()
    return nc


def _get_nc():
    if "nc" not in _CACHE:
        _CACHE["nc"] = _build_bass()
    return _CACHE["nc"]


def _run(x, wa, wb, wt, trace=False, **spmd_kwargs):
    from concourse import bass_utils

    nc = _get_nc()
    x = np.asarray(x, dtype=np.float32).astype(ml_dtypes.bfloat16)
    wa = np.asarray(wa, dtype=np.float32)
    wb = np.asarray(wb, dtype=np.float32)
    wt = np.asarray(wt, dtype=np.float32)
    eat, ebt, coef = _host_prep(wa, wb, wt)

    in_maps = []
    for c in range(N_CORES):
        in_maps.append({
            "xs": np.ascontiguousarray(x[:, c * BSH:(c + 1) * BSH]),
            "eat": eat, "ebt": ebt, "coef": coef,
        })
    res = bass_utils.run_bass_kernel_spmd(nc, in_maps, core_ids=list(range(N_CORES)),
                                          trace=trace, **spmd_kwargs)
    out = np.concatenate(
        [res.results[c]["out"].astype(np.float32) for c in range(N_CORES)], axis=1)
    return out, res


def kernel(x, wa, wb, wt):
    out, _ = _run(x, wa, wb, wt, trace=False)
    return out


# revision 8
# speedup vs baseline: 1.2761x; 1.0682x over previous
"""Trainium2 Bass kernel for the fused soft-logic-gate layer.

Reference computation:
    pa = softmax(wa, axis=1); pb = softmax(wb, axis=1); pt = softmax(wt, axis=0)
    A = pa @ x; B = pb @ x
    out = sum_g pt[g,:,None] * gate_g(A, B)        (16 soft logic gates)

Every gate is affine in {1, A, B, A*B}, so the 16-gate table collapses to
    out = c0 + cA*A + cB*B + cAB*(A*B)
with per-row coefficients derived from pt.  All weight-only math (exp,
transposes, softmax denominators, the coefficient chain) is precomputed on
the host in float64; x and the exp-weights are cast to bf16 on the host
(tolerance is 2e-2; bf16 contributes ~3e-3 and halves both HBM traffic and
matmul time, and enables the PE fast-weight-load path).  The device
computes, per batch tile:
    PSUM:  A = ea^T @ x ; B = eb^T @ x          (TensorE, bf16 -> f32)
    ACT:   s = ss*B + ca                        (scalar engine)
    DVE:   p = (A + u) * s                      (scalar_tensor_tensor)
    o = ro*p + w  -> bf16                       (DVE for m=0, ACT for m=1)
using the numerically-safe factoring  out = (A_t+U)(cAB_t*B_t+cA_t)+W  with
the normalizations folded in (ss=cAB_t/rsb, ca=cA_t, u=U*rsa, ro=1/rsa,
w=W); the f32 PSUM/intermediates keep the near-singular-cAB rows exact.
Output is stored bf16 and upcast to f32 on the host.

Schedule notes: all DMAs ride the SP hardware-DGE queue, inputs enqueued
first; B's matmuls precede A's so the scalar-engine `s` op overlaps A's
matmuls and PSUM banks recycle without stalling the PE; a short burst of
throwaway matmuls on memset scratch ramps the PE out of its low-clock
p-state during the DMA head; the final tile's epilogue runs in half-tiles
to shorten the drain, and epilogue affine ops alternate DVE/ACT to balance
the two engines.

Sharding: batch axis of x split evenly across 8 NeuronCores (data parallel),
weights replicated.
"""

import os
import sys

for _p in ("/opt/trn_rl_repo",):
    if _p not in sys.path and os.path.isdir(_p):
        sys.path.insert(0, _p)

import numpy as np
import ml_dtypes

SIZE = 256
PREV = 256
BATCH = 32768
N_CORES = 8
BSH = BATCH // N_CORES  # per-core batch shard
CH = 1024               # chunk width (2 PSUM banks per mat)
NCH = BSH // CH
P = 128
NWARM = 8               # PE p-state warm-up matmuls

_CACHE = {}


def _sign_matrix() -> np.ndarray:
    """[16,5] f64 columns: [colsum, c0, cA, cB, cAB] — gate-table
    coefficients of {1, A, B, A*B} preceded by the softmax denominator."""
    S = np.zeros((16, 5), dtype=np.float64)
    S[:, 0] = 1.0
    S[8:16, 1] = 1.0
    for g in (2, 3, 6, 7):
        S[g, 2] += 1.0
    for g in (8, 9, 12, 13):
        S[g, 2] -= 1.0
    for g in (4, 5, 6, 7):
        S[g, 3] += 1.0
    for g in (8, 9, 10, 11):
        S[g, 3] -= 1.0
    for g, v in {1: 1, 2: -1, 4: -1, 6: -2, 7: -1, 8: 1, 9: 2, 11: 1, 13: 1, 14: -1}.items():
        S[g, 4] = v
    return S


def _host_prep(wa, wb, wt):
    """f64 weight-only preprocessing -> (eat, ebt, coef) device arrays."""
    wa = wa.astype(np.float64)
    wb = wb.astype(np.float64)
    wt = wt.astype(np.float64)
    ea = np.exp(wa)                      # [size, prev]
    eb = np.exp(wb)
    # the matmuls run on bf16-rounded weights; fold the matching row sums
    # packed [128, 2*size] layout (2KB DMA lines): row p holds
    # [k-block 0: all m, k-block 1: all m] of exp(w)^T
    eat = ea.T.astype(ml_dtypes.bfloat16)   # [prev, size]
    ebt = eb.T.astype(ml_dtypes.bfloat16)
    rsa = eat.astype(np.float64).sum(axis=0)
    rsb = ebt.astype(np.float64).sum(axis=0)
    cps = np.exp(wt).T @ _sign_matrix()  # [size, 5]
    Ssum, c0n, cAn, cBn, cABn = cps.T
    # normalized gate coefficients
    c0 = c0n / Ssum
    cA = cAn / Ssum
    cB = cBn / Ssum
    cAB = cABn / Ssum
    U = cB / cAB
    W = c0 - cA * U
    # device-side per-row scalars: s = ss*B + ca ; p = (A+u)*s ; o = ro*p + w
    ss = cAB / rsb
    ca = cA
    u = U * rsa
    ro = 1.0 / rsa
    w = W
    coef = np.stack([u, ss, ca, ro, w], axis=0)          # [5, 256]
    coef = coef.reshape(5, 2, P).transpose(2, 0, 1)      # [128, 5, 2]
    coef = np.ascontiguousarray(coef.reshape(P, 10), dtype=np.float32)
    eat = np.ascontiguousarray(eat.reshape(2, P, SIZE).transpose(1, 0, 2).reshape(P, 2 * SIZE))
    ebt = np.ascontiguousarray(ebt.reshape(2, P, SIZE).transpose(1, 0, 2).reshape(P, 2 * SIZE))
    return eat, ebt, coef


def _build_bass():
    import concourse.bacc as bacc
    import concourse.tile as tile
    import concourse.mybir as mybir

    f32 = mybir.dt.float32
    bf16 = mybir.dt.bfloat16
    Act = mybir.ActivationFunctionType
    Alu = mybir.AluOpType

    nc = bacc.Bacc(trn_type="TRN2", target_bir_lowering=False, debug=False,
                   num_devices=N_CORES)

    xs_d = nc.dram_tensor("xs", [PREV, BSH], bf16, kind="ExternalInput").ap()
    eat_d = nc.dram_tensor("eat", [P, 2 * SIZE], bf16, kind="ExternalInput").ap()
    ebt_d = nc.dram_tensor("ebt", [P, 2 * SIZE], bf16, kind="ExternalInput").ap()
    coef_d = nc.dram_tensor("coef", [P, 10], f32, kind="ExternalInput").ap()
    out_d = nc.dram_tensor("out", [SIZE, BSH], bf16, kind="ExternalOutput").ap()

    # [p, k, b] view for single-DMA transfers
    xs_v = xs_d.rearrange("(k p) b -> p k b", p=P)
    eat_v = eat_d.rearrange("p (k m) -> p k m", k=2)
    ebt_v = ebt_d.rearrange("p (k m) -> p k m", k=2)

    with tile.TileContext(nc) as tc:
        with tc.tile_pool(name="consts", bufs=1) as consts, \
             tc.tile_pool(name="ep", bufs=3) as ep:

            # PE p-state warm-up on scratch SBUF (values irrelevant)
            scratch = consts.tile([P, 256], bf16, tag="scratch")
            nc.gpsimd.memset(scratch[:], 1.0)

            # input DMAs in dependency order on the SP (hwdge) queue;
            # last chunk's x split in two for a shorter drain
            xtiles = [consts.tile([P, 2, CH], bf16, tag=f"x{t}", name=f"x{t}")
                      for t in range(NCH - 1)]
            xlast = [consts.tile([P, 2, 512], bf16, tag=f"xl{h}", name=f"xl{h}")
                     for h in range(2)]
            nc.sync.dma_start(out=xtiles[0][:], in_=xs_v[:, :, 0:CH])
            ebT = consts.tile([P, 2, SIZE], bf16, tag="ebT")
            nc.sync.dma_start(out=ebT[:], in_=ebt_v[:])
            coef = consts.tile([P, 10], f32, tag="coef")
            nc.sync.dma_start(out=coef[:], in_=coef_d[:])
            eaT = consts.tile([P, 2, SIZE], bf16, tag="eaT")
            nc.sync.dma_start(out=eaT[:], in_=eat_v[:])
            for t in range(1, NCH - 1):
                nc.sync.dma_start(out=xtiles[t][:],
                                  in_=xs_v[:, :, t * CH:(t + 1) * CH])
            for h in range(2):
                base = (NCH - 1) * CH + h * 512
                nc.sync.dma_start(out=xlast[h][:],
                                  in_=xs_v[:, :, base:base + 512])

            cv = coef[:].rearrange("p (c m) -> p c m", c=5)
            u2, ss2, ca2, ro2, w2 = (cv[:, i, :] for i in range(5))

            with tc.tile_pool(name="warm_ps", bufs=1, space="PSUM") as warm_ps:
                wps = warm_ps.tile([P, 256], f32, tag="wps")
                for r in range(NWARM):
                    nc.tensor.matmul(wps[:], scratch[:, 0:P], scratch[:],
                                     start=True, stop=True, skip_group_check=True)

            def xslab(n, s):
                if n == NCH - 1:
                    return xlast[s]
                return xtiles[n][:, :, s * 512:(s + 1) * 512]

            # ---- main loop ----
            with tc.tile_pool(name="mm_ps", bufs=2, space="PSUM") as mm_ps:
                for n in range(NCH):
                    for m in range(2):
                        final = n == NCH - 1 and m == 1
                        b_ps = mm_ps.tile([P, CH], f32, tag="B", name=f"B{n}{m}")
                        a_ps = mm_ps.tile([P, CH], f32, tag="A", name=f"A{n}{m}")
                        # B first: the s-ACT consumes it while A's matmuls run
                        for s in range(CH // 512):
                            sl = slice(s * 512, (s + 1) * 512)
                            xt = xslab(n, s)
                            for k in range(2):
                                nc.tensor.matmul(b_ps[:, sl],
                                                 ebT[:, k, m * P:(m + 1) * P],
                                                 xt[:, k, :],
                                                 start=(k == 0), stop=(k == 1))
                        s_sb = ep.tile([P, CH], f32, tag="s", name=f"s{n}{m}")
                        s_halves = (slice(0, 512), slice(512, CH)) if final else (slice(0, CH),)
                        for hl in s_halves:
                            nc.scalar.activation(out=s_sb[:, hl], in_=b_ps[:, hl],
                                                 func=Act.Identity,
                                                 scale=ss2[:, m:m + 1],
                                                 bias=ca2[:, m:m + 1])
                        for s in range(CH // 512):
                            sl = slice(s * 512, (s + 1) * 512)
                            xt = xslab(n, s)
                            for k in range(2):
                                nc.tensor.matmul(a_ps[:, sl],
                                                 eaT[:, k, m * P:(m + 1) * P],
                                                 xt[:, k, :],
                                                 start=(k == 0), stop=(k == 1))
                        p_sb = ep.tile([P, CH], f32, tag="p", name=f"p{n}{m}")
                        o_sb = ep.tile([P, CH], bf16, tag="o", name=f"o{n}{m}")
                        for hl in s_halves:
                            nc.vector.scalar_tensor_tensor(out=p_sb[:, hl],
                                                           in0=a_ps[:, hl],
                                                           scalar=u2[:, m:m + 1],
                                                           in1=s_sb[:, hl],
                                                           op0=Alu.add, op1=Alu.mult)
                            # o = ro*p + w: DVE for m=0, ACT for m=1
                            if m == 0:
                                nc.vector.tensor_scalar(out=o_sb[:, hl], in0=p_sb[:, hl],
                                                        scalar1=ro2[:, m:m + 1],
                                                        scalar2=w2[:, m:m + 1],
                                                        op0=Alu.mult, op1=Alu.add)
                            else:
                                nc.scalar.activation(out=o_sb[:, hl], in_=p_sb[:, hl],
                                                     func=Act.Identity,
                                                     scale=ro2[:, m:m + 1],
                                                     bias=w2[:, m:m + 1])
                            nc.sync.dma_start(
                                out=out_d[m * P:(m + 1) * P,
                                          n * CH + hl.start:n * CH + hl.stop],
                                in_=o_sb[:, hl])

    nc.compile()
    return nc


def _get_nc():
    if "nc" not in _CACHE:
        _CACHE["nc"] = _build_bass()
    return _CACHE["nc"]


def _run(x, wa, wb, wt, trace=False, **spmd_kwargs):
    from concourse import bass_utils

    nc = _get_nc()
    x = np.asarray(x, dtype=np.float32).astype(ml_dtypes.bfloat16)
    wa = np.asarray(wa, dtype=np.float32)
    wb = np.asarray(wb, dtype=np.float32)
    wt = np.asarray(wt, dtype=np.float32)
    eat, ebt, coef = _host_prep(wa, wb, wt)

    in_maps = []
    for c in range(N_CORES):
        in_maps.append({
            "xs": np.ascontiguousarray(x[:, c * BSH:(c + 1) * BSH]),
            "eat": eat, "ebt": ebt, "coef": coef,
        })
    res = bass_utils.run_bass_kernel_spmd(nc, in_maps, core_ids=list(range(N_CORES)),
                                          trace=trace, **spmd_kwargs)
    out = np.concatenate(
        [res.results[c]["out"].astype(np.float32) for c in range(N_CORES)], axis=1)
    return out, res


def kernel(x, wa, wb, wt):
    out, _ = _run(x, wa, wb, wt, trace=False)
    return out
